# revision 30
# baseline (speedup 1.0000x reference)
"""Bass/Trainium2 kernel for nn_DualSignalLearning (8-core SPMD).

Sharding: node dim N=64 split 8 ways (8 local nodes/core). Small batch
quantities (mse, T_v, Tbar) are computed redundantly per core. Per-(node,f)
contribution norms and per-node spectral norms are AllGathered (8x40 floats)
so every core can run the 64-step reverse-topological coefficient recurrence,
which collapses to c = (I + M_low)(I - M_up)^{-1} e_{N-1} because every
tension row stays a scalar multiple of T_v.

Spectral norm: the reference's 12 D-space power iterations collapse to
R-space using the Grams G_A = A^T A, G_B = B^T B (H = G_B G_A); iterates are
trace-normalized (scale cancels in the final Rayleigh ratio) to avoid
under/overflow.

Adam with m=v=0 inputs reduces exactly to W - eta*g/(|g|+eps); a general
(m,v != 0) variant is compiled lazily if those inputs are ever nonzero.
"""
import sys
import numpy as np
from contextlib import ExitStack

for _p in ("/opt/trn_rl_repo",):
    if _p not in sys.path:
        sys.path.insert(0, _p)

import concourse.bass as bass
import concourse.tile as tile
from concourse import bacc, mybir
from concourse.bass_utils import run_bass_kernel_spmd

FP = mybir.dt.float32
N, F, B, D, R = 64, 4, 32, 1024, 64
NC, NL = 8, 8                      # cores, local nodes per core
ETA_W, ETA_S = 0.015, 0.003
LAM_C, LAM_R = 0.65, 0.35
THETA0 = 0.5
SPEC_MIN, SPEC_MAX = 0.3, 4.0
B1, B2, EPS = 0.9, 0.999, 1e-8
W_MAX = 2.0 * float(np.sqrt(D))
ALPHA, BETA_H = 0.01, 0.05

Alu = mybir.AluOpType
Act = mybir.ActivationFunctionType
AxX = mybir.AxisListType.X

_BUILT = {}


def _build(general_adam: bool, sim: bool = False):
    key = (general_adam, sim)
    if key in _BUILT:
        return _BUILT[key]
    nc = bacc.Bacc("TRN2", target_bir_lowering=False, debug=False, num_devices=NC)

    def inp(name, shape):
        return nc.declare_dram_parameter(name, list(shape), FP, isOutput=False)

    def outp(name, shape):
        return nc.declare_dram_parameter(name, list(shape), FP, isOutput=True)

    yh_d = inp("yh", (B, D))
    ys_d = inp("ys", (B, D))
    ctr_d = inp("ctr", (NL * F * B, D))        # contribs shard, row (n,f,b)
    vin_d = inp("vin", (NL * B, D))
    vout_d = inp("vout", (NL * B, D))
    vw_d = inp("vw", (NL * B, D))
    a_d = inp("amat", (NL, D, R))
    b_d = inp("bmat", (NL, D, R))
    if general_adam:
        ma_d = inp("ma", (NL, D, R))
        va_d = inp("va", (NL, D, R))
        mb_d = inp("mb", (NL, D, R))
        vb_d = inp("vb", (NL, D, R))
    gd_d = inp("gd", (NL, 1))
    rho_d = inp("rhoi", (1, NL))
    rdot_d = inp("rdoti", (1, NL))
    s_d = inp("smat", (NL, D))
    # host-prepared constants, packed into two blobs (2 DMAs total)
    blobA_d = inp("blobA", (128, 145))
    blobB_d = inp("blobB", (64, 1147))

    anew_d = outp("anew", (NL, D, R))
    bnew_d = outp("bnew", (NL, D, R))
    tens_d = outp("tens", (NL, B, D))
    snew_d = outp("snew", (NL, D))
    rhoo_d = outp("rhoo", (1, NL))
    mse_d = outp("mseo", (1, 1))
    dbgc_d = outp("dbgc", (N, 1))
    dbgu_d = outp("dbgu", (NL, D))

    cc_in = nc.dram_tensor("cc_in", [NL, 5], FP)
    cc_out = nc.dram_tensor("cc_out", [N, 5], FP, addr_space="Shared")

    with tile.TileContext(nc) as tc, ExitStack() as ctx:
        cpool = ctx.enter_context(tc.tile_pool(name="consts", bufs=1))
        main = ctx.enter_context(tc.tile_pool(name="main", bufs=1))
        strm = ctx.enter_context(tc.tile_pool(name="strm", bufs=3))
        scr = ctx.enter_context(tc.tile_pool(name="scr", bufs=3))
        sm = ctx.enter_context(tc.tile_pool(name="small", bufs=2))
        spool = ctx.enter_context(tc.tile_pool(name="spec", bufs=1))
        # PSUM: 8 banks total. pb:2 + pm:2 + pv:3 + acc:1 = 8.
        ppb = ctx.enter_context(tc.tile_pool(name="ppb", bufs=2, space="PSUM"))
        ppm = ctx.enter_context(tc.tile_pool(name="ppm", bufs=3, space="PSUM"))
        ppv = ctx.enter_context(tc.tile_pool(name="ppv", bufs=2, space="PSUM"))
        pacc = ctx.enter_context(tc.tile_pool(name="pacc", bufs=1, space="PSUM"))

        def pb(shape):
            return ppb.tile(list(shape), FP, tag="pb", name="pbt")

        def pm(shape):
            return ppm.tile(list(shape), FP, tag="pm", name="pmt")

        def pv(shape):
            return ppv.tile(list(shape), FP, tag="pv", name="pvt")

        blobA = cpool.tile([128, 145], FP, tag="blobA")
        nc.scalar.dma_start(blobA[:], blobA_d[:])
        blobB = cpool.tile([64, 1147], FP, tag="blobB")
        nc.scalar.dma_start(blobB[:], blobB_d[:])
        ident = blobA[:, 0:128]
        selv8 = [blobA[:, 128:136], blobA[:, 136:144]]
        o128 = blobA[:, 144:145]
        eupT = blobB[:, 0:256]
        elowT = blobB[:, 256:512]
        seln = blobB[:, 512:520]
        e63 = blobB[:, 520:521]
        o64c = blobB[:, 521:522]
        o1_64 = blobB[0:1, 522:586]
        o1_32 = blobB[0:1, 586:618]
        selb = blobB[0:32, 618:626]
        o32a = blobB[0:32, 626:627]
        o32b = blobB[0:32, 627:628]
        mask8 = blobB[0:NL, 628:632]
        lcm = blobB[0:NL, 632:633]
        lrm = blobB[0:NL, 633:634]
        nm8 = blobB[0:NL, 634:635]
        blockmask = blobB[0:NL, 635:1147]

        def tp(out_ps, in_sb):
            k = in_sb.partition_size()
            b = in_sb.base_partition()
            nc.tensor.transpose(out_ps, in_sb, ident[b:b + k, b:b + k])

        # A|B blocks resident in SBUF: abblk[t] is (128, 1024),
        # node n occupies cols [128n,128n+128) as [B_n(64) | A_n(64)].
        abblk = []
        for t in range(8):
            tl = main.tile([128, 1024], FP, tag=f"ab{t}")
            pair = tl[:].rearrange("p (n c) -> p n c", n=NL)
            src_b = b_d[:, 128 * t:128 * (t + 1), :].transpose([1, 0, 2])
            src_a = a_d[:, 128 * t:128 * (t + 1), :].transpose([1, 0, 2])
            nc.sync.dma_start(pair[:, :, 0:64], src_b)
            nc.sync.dma_start(pair[:, :, 64:128], src_a)
            abblk.append(tl)

        def acol(t, n):
            return abblk[t][:, 128 * n + 64:128 * n + 128]

        def bcol(t, n):
            return abblk[t][:, 128 * n:128 * n + 64]

        def abpair(t, n):
            return abblk[t][:, 128 * n:128 * n + 128]

        # ---- phase 1: diff / mse / Tbar / Tl_base -------------------------
        yh = scr.tile([B, D], FP, tag="scratch")
        ys = scr.tile([B, D], FP, tag="scratch")
        nc.scalar.dma_start(yh[:], yh_d[:])
        nc.scalar.dma_start(ys[:], ys_d[:])
        diff = main.tile([B, D], FP, tag="diff")
        nc.vector.tensor_tensor(diff[:], ys[:], yh[:], Alu.subtract)
        dsq = scr.tile([B, D], FP, tag="scratch")
        dacc = sm.tile([B, 1], FP, tag="dacc")
        nc.scalar.activation(dsq[:], diff[:], Act.Square, accum_out=dacc[:])
        mse_ps = pv((1, 1))
        nc.tensor.matmul(mse_ps[:], dacc[:], o32a[:])
        mse_sb = sm.tile([1, 1], FP, tag="mse_sb")
        nc.vector.tensor_copy(mse_sb[:], mse_ps[:])
        nc.sync.dma_start(mse_d[:], mse_sb[:])
        sqc = sm.tile([B, 1], FP, tag="sqc")
        nc.scalar.activation(sqc[:], dacc[:], Act.Sqrt)
        tl_ps = pv((1, 1))
        nc.tensor.matmul(tl_ps[:], sqc[:], o32b[:])
        tl_sb = sm.tile([1, 1], FP, tag="tl_sb")     # Tl_base
        nc.vector.tensor_copy(tl_sb[:], tl_ps[:])
        tnraw = sm.tile([1, 1], FP, tag="tnraw")
        nc.scalar.activation(tnraw[:], mse_sb[:], Act.Sqrt)
        tn = sm.tile([1, 1], FP, tag="tn")           # T_norm
        nc.vector.tensor_scalar_min(tn[:], tnraw[:], 1.0)
        tbar8 = main.tile([NL, D], FP, tag="tbar8")  # Tbar replicated x8
        for ch in range(2):
            cols = slice(512 * ch, 512 * (ch + 1))
            tb_ps = pb((NL, 512))
            nc.tensor.matmul(tb_ps[:], selb[:], diff[:, cols])
            nc.scalar.copy(tbar8[:, cols], tb_ps[:])

        # ---- phase 2: contribution norms ---------------------------------
        sumsqs = main.tile([128, NL], FP, tag="sumsqs")
        for n in range(NL):
            ctile = strm.tile([128, D], FP, tag="ctile", bufs=2)
            nc.scalar.dma_start(ctile[:], ctr_d[128 * n:128 * (n + 1), :])
            csq = scr.tile([128, D], FP, tag="scratch")
            nc.scalar.activation(csq[:], ctile[:], Act.Square,
                                 accum_out=sumsqs[:, n:n + 1])
        nrmc = main.tile([128, NL], FP, tag="nrmc")
        nc.scalar.activation(nrmc[:], sumsqs[:], Act.Sqrt)
        nrmt_ps = pm((NL, 128))
        tp(nrmt_ps[:], nrmc[:])
        nrm8 = main.tile([NL, 128], FP, tag="nrm8")
        nc.vector.tensor_copy(nrm8[:], nrmt_ps[:])
        ccsb = main.tile([NL, 5], FP, tag="ccsb")
        red = sm.tile([NL, F], FP, tag="red")
        nc.vector.tensor_reduce(red[:], nrm8[:].rearrange("p (f b) -> p f b", f=F),
                                axis=AxX, op=Alu.add)
        nc.vector.tensor_tensor(ccsb[:, 0:4], red[:], mask8[:], Alu.mult)

        # ---- phase 3: Grams + spectral norm (node-batched tiles) ---------
        # all per-node (64,64) matrices live side-by-side in (64,512) tiles;
        # each level: 8 matmuls into one PSUM bank + ONE copy out.
        def mcol(tile_, n):
            return tile_[:, 64 * n:64 * (n + 1)]

        # pair-Gram: ONE matmul [B|A]^T [B|A] per (node, t): G_B sits at
        # rows 0:64 cols 0:64, G_A at rows 64:128 cols 64:128 (G_A is
        # extracted to base-0 partitions via PE transpose; exact symmetry).
        ga = spool.tile([N, 512], FP, tag="ga")
        gb = spool.tile([N, 512], FP, tag="gb")
        gatmp = spool.tile([128, 256], FP, tag="gatmp")
        p0 = spool.tile([N, NL], FP, tag="p0")
        p0_ps = pv((N, NL))
        for half in range(2):
            pair_ps = pm((128, 512))
            for k in range(4):
                n = 4 * half + k
                for t in range(8):
                    nc.tensor.matmul(pair_ps[:, 128 * k:128 * (k + 1)],
                                     abpair(t, n), abpair(t, n),
                                     start=(t == 0), stop=(t == 7))
            dstb = gb[:, 256 * half:256 * (half + 1)].rearrange(
                "p (k r) -> p k r", k=4)
            nc.any.tensor_copy(
                dstb, pair_ps[0:64].rearrange("p (k c) -> p k c", k=4)[:, :, 0:64])
            nc.any.tensor_copy(
                gatmp[64:128, :].rearrange("p (k r) -> p k r", k=4),
                pair_ps[64:128].rearrange("p (k c) -> p k c", k=4)[:, :, 64:128])
            gat_ps = pm((N, 256))
            for k in range(4):
                tp(gat_ps[:, 64 * k:64 * (k + 1)],
                   gatmp[64:128, 64 * k:64 * (k + 1)])
            nc.any.tensor_copy(ga[:, 256 * half:256 * (half + 1)], gat_ps[:])
        for n in range(NL):
            for t in range(8):
                nc.tensor.matmul(p0_ps[:, n:n + 1], bcol(t, n), o128,
                                 start=(t == 0), stop=(t == 7))
        nc.vector.tensor_copy(p0[:], p0_ps[:])

        hT_ps = pm((N, 512))
        for n in range(NL):
            nc.tensor.matmul(mcol(hT_ps, n), mcol(ga, n), mcol(gb, n))
        # batched trace normalization: 1/(trace_n/8 + tiny)
        dm = spool.tile([N, 512], FP, tag="dm")
        nc.vector.tensor_tensor(
            dm[:].rearrange("p (n r) -> p n r", n=NL),
            hT_ps[:].rearrange("p (n r) -> p n r", n=NL),
            ident[0:N, 0:N].unsqueeze(1).broadcast_to((N, NL, N)), Alu.mult)
        dred = spool.tile([N, NL], FP, tag="dred")
        nc.vector.tensor_reduce(dred[:], dm[:].rearrange("p (n r) -> p n r", n=NL),
                                axis=AxX, op=Alu.add)
        tr_ps = pv((1, NL))
        nc.tensor.matmul(tr_ps[:], o64c, dred[:])
        tr_row = sm.tile([1, NL], FP, tag="tr_row")
        nc.vector.tensor_scalar_add(tr_row[:], tr_ps[:], 1e-30)
        rec_row = sm.tile([1, NL], FP, tag="rec_row")
        nc.vector.reciprocal(rec_row[:], tr_row[:])
        recb_ps = pv((N, NL))
        nc.tensor.matmul(recb_ps[:], o1_64[:], rec_row[:])
        recB = spool.tile([N, NL], FP, tag="recB")
        nc.vector.tensor_copy(recB[:], recb_ps[:])
        h1T = spool.tile([N, 512], FP, tag="h1T")
        nc.vector.tensor_tensor(
            h1T[:].rearrange("p (n r) -> p n r", n=NL),
            hT_ps[:].rearrange("p (n r) -> p n r", n=NL),
            recB[:].unsqueeze(2).broadcast_to((N, NL, N)), Alu.mult)

        h1P_ps = pm((N, 512))
        for n in range(NL):
            tp(mcol(h1P_ps, n), mcol(h1T, n))
        h1P = spool.tile([N, 512], FP, tag="h1P")
        nc.scalar.copy(h1P[:], h1P_ps[:])

        def sq_level(Ps, Ts, tagT, tagP, need_p=True):
            t_ps = pm((N, 512))
            for n in range(NL):
                nc.tensor.matmul(mcol(t_ps, n), mcol(Ps, n), mcol(Ts, n))
            Tn = spool.tile([N, 512], FP, tag=tagT, name=tagT)
            nc.scalar.copy(Tn[:], t_ps[:])
            Pn = None
            if need_p:
                p_ps = pm((N, 512))
                for n in range(NL):
                    nc.tensor.matmul(mcol(p_ps, n), mcol(Ts, n), mcol(Ps, n))
                Pn = spool.tile([N, 512], FP, tag=tagP, name=tagP)
                nc.scalar.copy(Pn[:], p_ps[:])
            return Tn, Pn

        h2T, h2P = sq_level(h1P, h1T, "h2T", "h2P")
        h4T, h4P = sq_level(h2P, h2T, "h4T", "h4P")
        h8T, _ = sq_level(h4P, h4T, "h8T", "h8P", need_p=False)

        def matvec_level(mats, vecs, tag):
            mv_ps = pv((N, NL))
            for n in range(NL):
                nc.tensor.matmul(mv_ps[:, n:n + 1], mcol(mats, n), vecs[:, n:n + 1])
            out = spool.tile([N, NL], FP, tag=tag, name=tag)
            nc.vector.tensor_copy(out[:], mv_ps[:])
            return out

        v1 = matvec_level(h1T, p0, "v1")
        v2 = matvec_level(h2T, v1, "v2")
        v3 = matvec_level(h8T, v2, "v3")
        qv = matvec_level(ga, v3, "qv")
        rv = matvec_level(gb, qv, "rv")
        s1v = matvec_level(ga, rv, "s1v")
        nd_ps = pv((1, 2 * NL))
        for n in range(NL):
            nc.tensor.matmul(nd_ps[0:1, 2 * n:2 * n + 1], rv[:, n:n + 1], s1v[:, n:n + 1])
            nc.tensor.matmul(nd_ps[0:1, 2 * n + 1:2 * n + 2], qv[:, n:n + 1], rv[:, n:n + 1])
        numden = sm.tile([1, 2 * NL], FP, tag="numden")
        nc.vector.tensor_copy(numden[:], nd_ps[:])

        denr = sm.tile([1, NL], FP, tag="denr")
        nc.vector.tensor_scalar_add(denr[:], numden[:, 1::2], 1e-38)
        denri = sm.tile([1, NL], FP, tag="denri")
        nc.vector.reciprocal(denri[:], denr[:])
        sig2 = sm.tile([1, NL], FP, tag="sig2")
        nc.vector.tensor_tensor(sig2[:], numden[:, 0::2], denri[:], Alu.mult)
        sig = sm.tile([1, NL], FP, tag="sig")
        nc.scalar.activation(sig[:], sig2[:], Act.Sqrt)
        spec_row = sm.tile([1, NL], FP, tag="spec_row")
        nc.vector.tensor_scalar(spec_row[:], sig[:], SPEC_MIN, SPEC_MAX,
                                op0=Alu.max, op1=Alu.min)
        spec_ps = pv((NL, 1))
        tp(spec_ps[:], spec_row[:])
        nc.vector.tensor_copy(ccsb[:, 4:5], spec_ps[:])

        # ---- phase 4: AllGather ------------------------------------------
        nc.gpsimd.dma_start(cc_in[:], ccsb[:])
        if sim:
            # single-core TimelineSim build: stand in for the collective
            nc.gpsimd.dma_start(cc_out[0:NL], cc_in[:])
        else:
            nc.gpsimd.collective_compute(
                "AllGather", Alu.bypass, replica_groups=[list(range(NC))],
                ins=[cc_in[:]], outs=[cc_out[:]])
        gath = main.tile([N, 5], FP, tag="gath")
        nc.gpsimd.dma_start(gath[:], cc_out[:])

        # ---- phase 5: edge weights + coefficient chain -------------------
        den4 = sm.tile([N, 1], FP, tag="den4")
        nc.vector.tensor_reduce(den4[:], gath[:, 0:4], axis=AxX, op=Alu.add)
        den4e = sm.tile([N, 1], FP, tag="den4e")
        nc.vector.tensor_scalar_add(den4e[:], den4[:], 1e-9)
        recw = sm.tile([N, 1], FP, tag="recw")
        nc.vector.reciprocal(recw[:], den4e[:])
        wsp = sm.tile([N, F], FP, tag="wsp")
        nc.vector.tensor_scalar(wsp[:], gath[:, 0:4], recw[:], gath[:, 4:5],
                                op0=Alu.mult, op1=Alu.mult)

        def msum(esrc, tag):
            # w_all[i, f, t] = E[i, f, t] * wsp[i, f]; then sum over f via a
            # strided X-reduce (innermost dim = f, stride N)
            w_all = sm.tile([N, F * N], FP, tag="msw", name="msw", bufs=2)
            nc.vector.tensor_tensor(
                w_all[:].rearrange("p (f t) -> p f t", f=F),
                esrc[:].rearrange("p (f t) -> p f t", f=F),
                wsp[:].unsqueeze(2).broadcast_to((N, F, N)), Alu.mult)
            acc = sm.tile([N, N], FP, tag=f"m_{tag}")
            wv = bass.AP(w_all.tensor, w_all.offset,
                         [list(w_all.ap[0]), [1, N], [N, F]])
            nc.vector.tensor_reduce(acc[:], wv, axis=AxX, op=Alu.add)
            return acc

        mtup = msum(eupT, "up")      # (M_up)^T
        mtlow = msum(elowT, "low")   # (M_low)^T
        mup_ps = pm((N, N))
        tp(mup_ps[:], mtup[:])
        mup = sm.tile([N, N], FP, tag="mup")
        nc.scalar.copy(mup[:], mup_ps[:])

        # power series: v <- v + M^(2^j) v, with (P=M^k, T=(M^k)^T) pairs
        vcur = e63
        Pj, Tj = mup, mtup
        for j in range(6):
            mv_ps = pv((N, 1))
            nc.tensor.matmul(mv_ps[:], Tj[:], vcur[:])
            vnew = sm.tile([N, 1], FP, tag="cv", name="cv", bufs=2)
            nc.vector.tensor_tensor(vnew[:], mv_ps[:], vcur[:], Alu.add)
            vcur = vnew
            if j < 5:
                t_ps = pm((N, N))
                nc.tensor.matmul(t_ps[:], Pj[:], Tj[:])
                Tn = sm.tile([N, N], FP, tag="TpX", name="TpX", bufs=2)
                nc.scalar.copy(Tn[:], t_ps[:])
                if j < 4:
                    p_ps = pm((N, N))
                    nc.tensor.matmul(p_ps[:], Tj[:], Pj[:])
                    Pn = sm.tile([N, N], FP, tag="PpX", name="PpX", bufs=2)
                    nc.scalar.copy(Pn[:], p_ps[:])
                else:
                    Pn = None
                Pj, Tj = Pn, Tn
        low_ps = pv((N, 1))
        nc.tensor.matmul(low_ps[:], mtlow[:], vcur[:])
        c_sb = main.tile([N, 1], FP, tag="c_sb")
        nc.vector.tensor_tensor(c_sb[:], low_ps[:], vcur[:], Alu.add)
        nc.sync.dma_start(dbgc_d[:], c_sb[:])

        crow_ps = pv((1, NL))
        nc.tensor.matmul(crow_ps[:], c_sb[:], seln[:])
        c_row = sm.tile([1, NL], FP, tag="c_row")
        nc.vector.tensor_copy(c_row[:], crow_ps[:])
        cl_ps = pv((NL, 1))
        tp(cl_ps[:], c_row[:])
        c_loc = sm.tile([NL, 1], FP, tag="c_loc")
        nc.vector.tensor_copy(c_loc[:], cl_ps[:])
        cb_ps = pv((B, NL))
        nc.tensor.matmul(cb_ps[:], o1_32[:], c_row[:])
        cb = main.tile([B, NL], FP, tag="cb")
        nc.vector.tensor_copy(cb[:], cb_ps[:])

        # ---- phase 6: tensions out ---------------------------------------
        for n in range(NL):
            tt = strm.tile([B, D], FP, tag="ttile", bufs=2)
            nc.scalar.mul(tt[:], diff[:], cb[:, n:n + 1])
            nc.sync.dma_start(tens_d[n], tt[:])

        # ---- phase 7: health (rho) ---------------------------------------
        tloc = sm.tile([1, NL], FP, tag="tloc")
        nc.vector.tensor_scalar(tloc[:], c_row[:], tl_sb[0:1, 0:1], None,
                                op0=Alu.mult)
        tne = sm.tile([1, 1], FP, tag="tne")
        nc.vector.tensor_scalar_add(tne[:], tn[:], 1e-9)
        tnr = sm.tile([1, 1], FP, tag="tnr")
        nc.vector.reciprocal(tnr[:], tne[:])
        wb_pre = sm.tile([1, NL], FP, tag="wb_pre")
        nc.vector.tensor_scalar(wb_pre[:], tloc[:], tnr[0:1, 0:1], 1.0,
                                op0=Alu.mult, op1=Alu.min)
        fl = sm.tile([1, 1], FP, tag="fl")
        nc.vector.tensor_scalar(fl[:], tn[:], 0.0, None, op0=Alu.is_gt)
        wbm1 = sm.tile([1, NL], FP, tag="wbm1")
        nc.vector.tensor_scalar(wbm1[:], wb_pre[:], 1.0, None, op0=Alu.subtract)
        wbf = sm.tile([1, NL], FP, tag="wbf")
        nc.vector.tensor_scalar(wbf[:], wbm1[:], fl[0:1, 0:1], 1.0,
                                op0=Alu.mult, op1=Alu.add)
        k1 = sm.tile([1, 1], FP, tag="k1")
        nc.vector.tensor_scalar(k1[:], tn[:], -ALPHA, ALPHA, op0=Alu.mult,
                                op1=Alu.add)
        tsq = sm.tile([1, 1], FP, tag="tsq")
        nc.vector.tensor_tensor(tsq[:], tn[:], tn[:], Alu.mult)
        k2a = sm.tile([1, 1], FP, tag="k2a")
        nc.vector.tensor_scalar_add(k2a[:], tsq[:], 1.0)
        k2b = sm.tile([1, 1], FP, tag="k2b")
        nc.vector.tensor_tensor(k2b[:], k2a[:], tn[:], Alu.mult)
        k2 = sm.tile([1, 1], FP, tag="k2")
        nc.vector.tensor_scalar_mul(k2[:], k2b[:], BETA_H)
        negd = sm.tile([1, NL], FP, tag="negd")
        nc.vector.scalar_tensor_tensor(negd[:], wbf[:], k2[0:1, 0:1],
                                       k1[0:1, 0:1].broadcast_to((1, NL)),
                                       op0=Alu.mult, op1=Alu.subtract)
        deltac = sm.tile([1, NL], FP, tag="deltac")
        nc.vector.tensor_scalar(deltac[:], negd[:], -1.0, None, op0=Alu.mult)
        deltac2 = sm.tile([1, NL], FP, tag="deltac2")
        nc.vector.tensor_scalar(deltac2[:], deltac[:], 0.1, -0.1, op0=Alu.min,
                                op1=Alu.max)
        rho_t = sm.tile([1, NL], FP, tag="rho_t")
        nc.sync.dma_start(rho_t[:], rho_d[:])
        rho_n = sm.tile([1, NL], FP, tag="rho_n")
        nc.vector.tensor_tensor(rho_n[:], rho_t[:], deltac2[:], Alu.add)
        rho_c = sm.tile([1, NL], FP, tag="rho_c")
        nc.vector.tensor_scalar(rho_c[:], rho_n[:], 10.0, -5.0, op0=Alu.min,
                                op1=Alu.max)
        nc.sync.dma_start(rhoo_d[:], rho_c[:])

        # ---- phase 8: V statistics ---------------------------------------
        vmr = main.tile([NL, D], FP, tag="vmr")      # mean_b V_in
        vor = main.tile([NL, D], FP, tag="vor")      # mean_b V_out
        vor_q = main.tile([NL, D], FP, tag="vor_q")  # mean_b (V_out t^2)
        t2r = main.tile([NL, D], FP, tag="t2r")      # mean_b t^2
        vth_ps = pacc.tile([NL, 1], FP, tag="pacc")
        for g in range(2):
            vint = main.tile([128, D], FP, tag="vbig", name="vbig", bufs=4)
            voutt = main.tile([128, D], FP, tag="vbig", name="vbig", bufs=4)
            vwt = strm.tile([128, D], FP, tag="ctile", bufs=2)
            rows = slice(128 * g, 128 * (g + 1))
            nc.sync.dma_start(vint[:], vin_d[rows, :])
            nc.sync.dma_start(voutt[:], vout_d[rows, :])
            nc.sync.dma_start(vwt[:], vw_d[rows, :])
            th = scr.tile([128, D], FP, tag="scratch")
            nc.scalar.activation(th[:], vwt[:], Act.Tanh)
            t2t = main.tile([128, D], FP, tag="vbig", name="vbig", bufs=4)
            nc.vector.tensor_tensor(t2t[:], th[:], th[:], Alu.mult)
            qt = main.tile([128, D], FP, tag="vbig", name="vbig", bufs=4)
            nc.vector.tensor_tensor(qt[:], voutt[:], t2t[:], Alu.mult)
            vsq = scr.tile([128, D], FP, tag="scratch")
            vacc = sm.tile([128, 1], FP, tag=f"vacc{g}")
            nc.scalar.activation(vsq[:], vint[:], Act.Square, accum_out=vacc[:])
            nc.tensor.matmul(vth_ps[:], selv8[g][:], vacc[:], start=(g == 0),
                             stop=(g == 1))
            for dst, srct in ((vmr, vint), (vor, voutt), (vor_q, qt), (t2r, t2t)):
                for ch in range(2):
                    cols = slice(512 * ch, 512 * (ch + 1))
                    red_ps = pb((NL, 512))
                    nc.tensor.matmul(red_ps[:], selv8[g][:], srct[:, cols])
                    if g == 0:
                        nc.scalar.copy(dst[:, cols], red_ps[:])
                    else:
                        nc.vector.tensor_tensor(dst[:, cols], dst[:, cols],
                                                red_ps[:], Alu.add)

        theta = sm.tile([NL, 1], FP, tag="theta")
        nc.vector.tensor_scalar(theta[:], vth_ps[:], 1e-9, THETA0, op0=Alu.add,
                                op1=Alu.mult)
        gd_t = sm.tile([NL, 1], FP, tag="gd_t")
        nc.sync.dma_start(gd_t[:], gd_d[:])
        dgc = sm.tile([NL, 1], FP, tag="dgc")
        nc.vector.tensor_tensor(dgc[:], gd_t[:], theta[:], Alu.subtract)
        lamc_dg = sm.tile([NL, 1], FP, tag="lamc_dg")
        nc.vector.tensor_tensor(lamc_dg[:], dgc[:], lcm[:], Alu.mult)

        # ---- phase 9: u assembly ------------------------------------------
        qbar = scr.tile([NL, D], FP, tag="uscr")
        nc.vector.tensor_tensor(qbar[:], vor[:], vor_q[:], Alu.subtract)
        fpb = scr.tile([NL, D], FP, tag="uscr")
        nc.vector.tensor_scalar(fpb[:], t2r[:], -1.0, 1.0, op0=Alu.mult,
                                op1=Alu.add)
        zt = scr.tile([NL, D], FP, tag="uscr")
        nc.vector.tensor_tensor(zt[:], tbar8[:], fpb[:], Alu.mult)
        ucp = main.tile([NL, D], FP, tag="ucp")      # LAM_C*nmask*dg*qbar
        nc.vector.tensor_scalar(ucp[:], qbar[:], lamc_dg[:], None, op0=Alu.mult)
        ur0 = main.tile([NL, D], FP, tag="ur0")      # LAM_R*nmask*(Tbar.fpb)
        nc.vector.tensor_scalar(ur0[:], zt[:], lrm[:], None, op0=Alu.mult)
        u0 = main.tile([NL, D], FP, tag="u0")        # post-c: ucp + c*ur0
        nc.vector.scalar_tensor_tensor(u0[:], ur0[:], c_loc[:], ucp[:],
                                       op0=Alu.mult, op1=Alu.add)
        usq_s = scr.tile([NL, D], FP, tag="uscr")
        uacc = sm.tile([NL, 1], FP, tag="uacc")
        nc.scalar.activation(usq_s[:], u0[:], Act.Square, accum_out=uacc[:])
        vmsq_s = scr.tile([NL, D], FP, tag="uscr")
        vmacc = sm.tile([NL, 1], FP, tag="vmacc")
        nc.scalar.activation(vmsq_s[:], vmr[:], Act.Square, accum_out=vmacc[:])
        un = sm.tile([NL, 1], FP, tag="un")
        nc.scalar.activation(un[:], uacc[:], Act.Sqrt)
        vn = sm.tile([NL, 1], FP, tag="vn")
        nc.scalar.activation(vn[:], vmacc[:], Act.Sqrt)
        gn = sm.tile([NL, 1], FP, tag="gn")
        nc.vector.tensor_tensor(gn[:], un[:], vn[:], Alu.mult)
        gne = sm.tile([NL, 1], FP, tag="gne")
        nc.vector.tensor_scalar_add(gne[:], gn[:], 1e-12)
        rg = sm.tile([NL, 1], FP, tag="rg")
        nc.vector.reciprocal(rg[:], gne[:])
        s_col = sm.tile([NL, 1], FP, tag="s_col")
        nc.vector.tensor_scalar(s_col[:], rg[:], 5.0, 1.0, op0=Alu.mult,
                                op1=Alu.min)
        u_rows = main.tile([NL, D], FP, tag="u_rows")
        nc.vector.tensor_scalar(u_rows[:], u0[:], s_col[:], None, op0=Alu.mult)
        nc.sync.dma_start(dbgu_d[:], u_rows[:])

        # ---- phase 10: signature (S) --------------------------------------
        rd_t = sm.tile([1, NL], FP, tag="rd_t")
        nc.sync.dma_start(rd_t[:], rdot_d[:])
        rde = sm.tile([1, NL], FP, tag="rde")
        nc.vector.tensor_scalar_mul(rde[:], rd_t[:], ETA_S)
        s_tile = main.tile([NL, D], FP, tag="s_tile")
        nc.sync.dma_start(s_tile[:], s_d[:])
        s2 = main.tile([NL, D], FP, tag="s2")
        for ch in range(2):
            cols = slice(512 * ch, 512 * (ch + 1))
            so_ps = pb((NL, 512))
            nc.tensor.matmul(so_ps[:], rde[:], tbar8[0:1, cols])
            nc.vector.tensor_tensor(s2[:, cols], so_ps[:], s_tile[:, cols],
                                    Alu.add)
        s2sq = scr.tile([NL, D], FP, tag="uscr")
        sacc = sm.tile([NL, 1], FP, tag="sacc")
        nc.scalar.activation(s2sq[:], s2[:], Act.Square, accum_out=sacc[:])
        snrm = sm.tile([NL, 1], FP, tag="snrm")
        nc.scalar.activation(snrm[:], sacc[:], Act.Sqrt)
        srec = sm.tile([NL, 1], FP, tag="srec")
        nc.vector.reciprocal(srec[:], snrm[:])
        sfl = sm.tile([NL, 1], FP, tag="sfl")
        nc.vector.tensor_scalar(sfl[:], snrm[:], 1e-9, None, op0=Alu.is_gt)
        srm1 = sm.tile([NL, 1], FP, tag="srm1")
        nc.vector.tensor_scalar(srm1[:], srec[:], 1.0, None, op0=Alu.subtract)
        sf2 = sm.tile([NL, 1], FP, tag="sf2")
        nc.vector.tensor_tensor(sf2[:], sfl[:], srm1[:], Alu.mult)
        sfin = sm.tile([NL, 1], FP, tag="sfin")
        nc.vector.tensor_scalar_add(sfin[:], sf2[:], 1.0)
        s2n = scr.tile([NL, D], FP, tag="uscr")
        nc.vector.tensor_scalar(s2n[:], s2[:], sfin[:], None, op0=Alu.mult)
        sdlt = scr.tile([NL, D], FP, tag="uscr")
        nc.vector.tensor_tensor(sdlt[:], s2n[:], s_tile[:], Alu.subtract)
        sout = scr.tile([NL, D], FP, tag="uscr")
        nc.vector.scalar_tensor_tensor(sout[:], sdlt[:], nm8[:], s_tile[:],
                                       op0=Alu.mult, op1=Alu.add)
        nc.sync.dma_start(snew_d[:], sout[:])

        # ---- phase 11: transposes + matvecs (pre-collective where possible)
        # uv[t] columns: [0:8]=vm, [8:16]=ucp, [16:24]=ur0  (all c-free)
        uv = []
        for t in range(8):
            uvt = main.tile([128, 24], FP, tag=f"uv{t}")
            for j, rows_src in enumerate((vmr, ucp, ur0)):
                c_ps = pm((128, NL))
                tp(c_ps[:], rows_src[:, 128 * t:128 * (t + 1)])
                nc.vector.tensor_copy(uvt[:, 8 * j:8 * (j + 1)], c_ps[:])
            uv.append(uvt)

        # wa = B^T vm;  wb1 = A^T ucp;  wb2 = A^T ur0   (all c-free)
        watile = main.tile([N, NL], FP, tag="watile")
        wb1tile = main.tile([N, NL], FP, tag="wb1tile")
        wb2tile = main.tile([N, NL], FP, tag="wb2tile")
        wa_ps = pv((N, NL))
        wb1_ps = pv((N, NL))
        wb2_ps = pv((N, NL))
        for n in range(NL):
            for t in range(8):
                st = dict(start=(t == 0), stop=(t == 7))
                nc.tensor.matmul(wa_ps[:, n:n + 1], bcol(t, n),
                                 uv[t][:, n:n + 1], **st)
                nc.tensor.matmul(wb1_ps[:, n:n + 1], acol(t, n),
                                 uv[t][:, 8 + n:9 + n], **st)
                nc.tensor.matmul(wb2_ps[:, n:n + 1], acol(t, n),
                                 uv[t][:, 16 + n:17 + n], **st)
        nc.vector.tensor_copy(watile[:], wa_ps[:])
        nc.vector.tensor_copy(wb1tile[:], wb1_ps[:])
        nc.vector.tensor_copy(wb2tile[:], wb2_ps[:])
        war_ps = pv((NL, N))
        tp(war_ps[:], watile[:])
        wa_rows = main.tile([NL, N], FP, tag="wa_rows")
        nc.vector.tensor_copy(wa_rows[:], war_ps[:])
        wb1r_ps = pv((NL, N))
        tp(wb1r_ps[:], wb1tile[:])
        wb1_rows = main.tile([NL, N], FP, tag="wb1_rows")
        nc.vector.tensor_copy(wb1_rows[:], wb1r_ps[:])
        wb2r_ps = pv((NL, N))
        tp(wb2r_ps[:], wb2tile[:])
        wb2_rows = main.tile([NL, N], FP, tag="wb2_rows")
        nc.vector.tensor_copy(wb2_rows[:], wb2r_ps[:])
        # post-c: wb = s * (wb1 + c*wb2);  wa scaled by s is NOT applied to
        # wa (grad_A = u x wA keeps scaling inside u)
        wb_rows = main.tile([NL, N], FP, tag="wb_rows")
        nc.vector.scalar_tensor_tensor(wb_rows[:], wb2_rows[:], c_loc[:],
                                       wb1_rows[:], op0=Alu.mult, op1=Alu.add)
        wbs_rows = main.tile([NL, N], FP, tag="wbs_rows")
        nc.vector.tensor_scalar(wbs_rows[:], wb_rows[:], s_col[:], None,
                                op0=Alu.mult)

        # block-diagonal row tiles for the PE grad build:
        # bd[k, 64n+r] = [k==n] * w_rows[k, r]
        bd_wa = main.tile([NL, 512], FP, tag="bd_wa")
        nc.vector.tensor_tensor(
            bd_wa[:].rearrange("p (n r) -> p n r", n=NL),
            wa_rows[:].rearrange("p (g r) -> p g r", g=1
                                 ).broadcast_to((NL, NL, 64)),
            blockmask[:].rearrange("p (n r) -> p n r", n=NL), Alu.mult)
        bd_wb = main.tile([NL, 512], FP, tag="bd_wb")
        nc.vector.tensor_tensor(
            bd_wb[:].rearrange("p (n r) -> p n r", n=NL),
            wbs_rows[:].rearrange("p (g r) -> p g r", g=1
                                  ).broadcast_to((NL, NL, 64)),
            blockmask[:].rearrange("p (n r) -> p n r", n=NL), Alu.mult)

        # ---- phase 12: Adam updates ---------------------------------------
        for t in range(8):
            for side in range(2):  # 0: A-side (u x wA), 1: B-side (vm x wB)
                vec_rows = u_rows if side == 0 else vmr
                bd = bd_wa if side == 0 else bd_wb
                w_cols = (abblk[t][:].rearrange("p (n c) -> p n c", n=NL)
                          [:, :, 64:128] if side == 0 else
                          abblk[t][:].rearrange("p (n c) -> p n c", n=NL)
                          [:, :, 0:64])
                out_d = anew_d if side == 0 else bnew_d
                g_ps = pm((128, 512))
                nc.tensor.matmul(g_ps[:], vec_rows[:, 128 * t:128 * (t + 1)],
                                 bd[:])
                if general_adam:
                    g = strm.tile([128, 512], FP, tag="g", bufs=2)
                    nc.scalar.copy(g[:], g_ps[:])
                    mtile = strm.tile([128, 512], FP, tag="gA", name="gA", bufs=2)
                    vtile = strm.tile([128, 512], FP, tag="gB", name="gB", bufs=2)
                    m_d_, v_d_ = (ma_d, va_d) if side == 0 else (mb_d, vb_d)
                    nc.sync.dma_start(
                        mtile[:].rearrange("p (n r) -> p n r", n=NL),
                        m_d_[:, 128 * t:128 * (t + 1), :].transpose([1, 0, 2]))
                    nc.sync.dma_start(
                        vtile[:].rearrange("p (n r) -> p n r", n=NL),
                        v_d_[:, 128 * t:128 * (t + 1), :].transpose([1, 0, 2]))
                    mh = strm.tile([128, 512], FP, tag="mh", bufs=2)
                    nc.vector.scalar_tensor_tensor(
                        mh[:], mtile[:], B1 / (1.0 - B1), g[:],
                        op0=Alu.mult, op1=Alu.add)
                    g2 = strm.tile([128, 512], FP, tag="gA", name="gA", bufs=2)
                    nc.vector.tensor_tensor(g2[:], g[:], g[:], Alu.mult)
                    vh = strm.tile([128, 512], FP, tag="g", name="g", bufs=2)
                    nc.vector.scalar_tensor_tensor(
                        vh[:], vtile[:], B2 / (1.0 - B2), g2[:],
                        op0=Alu.mult, op1=Alu.add)
                    sq = strm.tile([128, 512], FP, tag="gA", name="gA", bufs=2)
                    nc.scalar.activation(sq[:], vh[:], Act.Sqrt)
                    den = strm.tile([128, 512], FP, tag="gB", name="gB", bufs=2)
                    nc.vector.tensor_scalar_add(den[:], sq[:], EPS)
                    rec = strm.tile([128, 512], FP, tag="rec", bufs=2)
                    nc.vector.reciprocal_approx_fast(rec[:], den[:])
                    u1 = strm.tile([128, 512], FP, tag="u1", bufs=2)
                    nc.vector.scalar_tensor_tensor(u1[:], mh[:], -ETA_W, rec[:],
                                                   op0=Alu.mult, op1=Alu.mult)
                else:
                    absg = strm.tile([128, 512], FP, tag="absg", bufs=2)
                    nc.scalar.activation(absg[:], g_ps[:], Act.Abs)
                    den = strm.tile([128, 512], FP, tag="den", bufs=2)
                    nc.vector.tensor_scalar_add(den[:], absg[:], EPS)
                    rec = strm.tile([128, 512], FP, tag="rec", bufs=2)
                    nc.vector.reciprocal_approx_fast(rec[:], den[:])
                    u1 = strm.tile([128, 512], FP, tag="u1", bufs=2)
                    nc.vector.scalar_tensor_tensor(u1[:], g_ps[:], -ETA_W,
                                                   rec[:], op0=Alu.mult,
                                                   op1=Alu.mult)
                upd = strm.tile([128, 512], FP, tag="upd", bufs=2)
                nc.vector.tensor_tensor(
                    upd[:].rearrange("p (n r) -> p n r", n=NL), u1[:].rearrange(
                        "p (n r) -> p n r", n=NL), w_cols, Alu.add)
                eng = nc.sync if side == 0 else nc.scalar
                eng.dma_start(
                    out_d[:, 128 * t:128 * (t + 1), :].transpose([1, 0, 2]),
                    upd[:].rearrange("p (n r) -> p n r", n=NL))

    nc.compile()
    _BUILT[key] = nc
    return nc


def _consts(core, src_ids, src_mask):
    """Host-prepared constant blobs for one core."""
    f32 = np.float32
    lo = core * NL
    A = np.zeros((128, 145), f32)
    A[:, 0:128] = np.eye(128, dtype=f32)
    for g in range(2):
        for p in range(128):
            A[p, 128 + 8 * g + 4 * g + p // 32] = 0  # placeholder (set below)
    # selv8: [p, n] = 1/32 if n == 4*g + p//32
    for g in range(2):
        blk = np.zeros((128, NL), f32)
        for p in range(128):
            blk[p, 4 * g + p // 32] = 1.0 / 32.0
        A[:, 128 + 8 * g:136 + 8 * g] = blk
    A[:, 144] = 1.0 / 32.0                      # o128 (p0 rhs, v0=1/32)

    Bb = np.zeros((64, 1147), f32)
    eup = np.zeros((N, F * N), f32)
    elow = np.zeros((N, F * N), f32)
    for i in range(N):
        for f in range(F):
            if not src_mask[i, f]:
                continue
            t = int(src_ids[i, f])
            if t < i:
                eup[i, N * f + t] += 1.0
            else:
                elow[i, N * f + t] += 1.0
    Bb[:, 0:256] = eup
    Bb[:, 256:512] = elow
    seln = np.zeros((N, NL), f32)
    for n in range(NL):
        seln[lo + n, n] = 1.0
    Bb[:, 512:520] = seln
    Bb[N - 1, 520] = 1.0                        # e63
    Bb[:, 521] = 1.0 / 8.0                      # o64c (trace/8)
    Bb[0, 522:586] = 1.0                        # o1_64
    Bb[0, 586:618] = 1.0 / 32.0                 # o1_32 (tensions bcast, /32)
    Bb[0:32, 618:626] = 1.0 / 1024.0            # selb (Tbar lhsT)
    Bb[0:32, 626] = 1.0 / 32768.0               # o32a (mse)
    Bb[0:32, 627] = 1.0 / 1024.0                # o32b (Tl_base)
    Bb[0:NL, 628:632] = src_mask[lo:lo + NL].astype(f32) / 32.0   # mask8
    nmask = (np.arange(lo, lo + NL) > 0).astype(f32)
    Bb[0:NL, 632] = LAM_C * nmask               # lcm
    Bb[0:NL, 633] = LAM_R * nmask               # lrm
    Bb[0:NL, 634] = nmask                       # nm8
    bm = np.zeros((NL, 512), f32)
    for n in range(NL):
        bm[n, 64 * n:64 * (n + 1)] = 1.0
    Bb[0:NL, 635:1147] = bm
    return {"blobA": A, "blobB": Bb}


def kernel(Y_hat, Y_star, contribs, V_in, V_out, V_weighted, goodness,
           A, B_w, m_A, v_A, m_B, v_B, S, rho, r_dot, src_ids, src_mask):
    f32 = np.float32
    Y_hat = np.ascontiguousarray(Y_hat, f32)
    Y_star = np.ascontiguousarray(Y_star, f32)
    contribs = np.ascontiguousarray(contribs, f32)
    V_in = np.ascontiguousarray(V_in, f32)
    V_out = np.ascontiguousarray(V_out, f32)
    V_weighted = np.ascontiguousarray(V_weighted, f32)
    goodness = np.ascontiguousarray(goodness, f32)
    A = np.ascontiguousarray(A, f32)
    B_w = np.ascontiguousarray(B_w, f32)
    S = np.ascontiguousarray(S, f32)
    rho = np.ascontiguousarray(rho, f32)
    r_dot = np.ascontiguousarray(r_dot, f32)
    src_ids = np.asarray(src_ids)
    src_mask = np.asarray(src_mask)

    general = any(bool(np.any(np.asarray(x))) for x in (m_A, v_A, m_B, v_B))
    nc = _build(general)

    in_maps = []
    for c in range(NC):
        lo = c * NL
        m = {
            "yh": Y_hat, "ys": Y_star,
            "ctr": contribs[lo:lo + NL].reshape(NL * F * B, D),
            "vin": V_in[lo:lo + NL].reshape(NL * B, D),
            "vout": V_out[lo:lo + NL].reshape(NL * B, D),
            "vw": V_weighted[lo:lo + NL].reshape(NL * B, D),
            "amat": A[lo:lo + NL], "bmat": B_w[lo:lo + NL],
            "gd": goodness[lo:lo + NL].reshape(NL, 1),
            "rhoi": rho[lo:lo + NL].reshape(1, NL),
            "rdoti": r_dot[lo:lo + NL].reshape(1, NL),
            "smat": S[lo:lo + NL],
        }
        if general:
            m["ma"] = m_A[lo:lo + NL]
            m["va"] = v_A[lo:lo + NL]
            m["mb"] = m_B[lo:lo + NL]
            m["vb"] = v_B[lo:lo + NL]
        m.update(_consts(c, src_ids, src_mask))
        in_maps.append({k: np.ascontiguousarray(v, f32) for k, v in m.items()})

    res = run_bass_kernel_spmd(nc, in_maps, list(range(NC)))
    outs = res.results

    mse = f32(outs[0]["mseo"][0, 0])
    A_new = np.concatenate([outs[c]["anew"] for c in range(NC)], 0)
    B_new = np.concatenate([outs[c]["bnew"] for c in range(NC)], 0)
    S_new = np.concatenate([outs[c]["snew"] for c in range(NC)], 0)
    rho_new = np.concatenate([outs[c]["rhoo"][0] for c in range(NC)], 0)
    tensions = np.concatenate([outs[c]["tens"] for c in range(NC)], 0)
    # clip is a no-op unless |A - eta*t| exceeds W_MAX; equivalent on host
    if np.abs(A_new).max() > W_MAX or np.abs(B_new).max() > W_MAX:
        np.clip(A_new, -W_MAX, W_MAX, out=A_new)
        np.clip(B_new, -W_MAX, W_MAX, out=B_new)
    return mse, A_new, B_new, S_new, rho_new, tensions


# revision 32
# speedup vs baseline: 1.0224x; 1.0224x over previous
"""Bass/Trainium2 kernel for nn_DualSignalLearning (8-core SPMD).

Sharding: node dim N=64 split 8 ways (8 local nodes/core). Small batch
quantities (mse, T_v, Tbar) are computed redundantly per core. Per-(node,f)
contribution norms and per-node spectral norms are AllGathered (8x40 floats)
so every core can run the 64-step reverse-topological coefficient recurrence,
which collapses to c = (I + M_low)(I - M_up)^{-1} e_{N-1} because every
tension row stays a scalar multiple of T_v.

Spectral norm: the reference's 12 D-space power iterations collapse to
R-space using the Grams G_A = A^T A, G_B = B^T B (H = G_B G_A); iterates are
trace-normalized (scale cancels in the final Rayleigh ratio) to avoid
under/overflow.

Adam with m=v=0 inputs reduces exactly to W - eta*g/(|g|+eps); a general
(m,v != 0) variant is compiled lazily if those inputs are ever nonzero.
"""
import sys
import numpy as np
from contextlib import ExitStack

for _p in ("/opt/trn_rl_repo",):
    if _p not in sys.path:
        sys.path.insert(0, _p)

import concourse.bass as bass
import concourse.tile as tile
from concourse import bacc, mybir
from concourse.bass_utils import run_bass_kernel_spmd

FP = mybir.dt.float32
N, F, B, D, R = 64, 4, 32, 1024, 64
NC, NL = 8, 8                      # cores, local nodes per core
ETA_W, ETA_S = 0.015, 0.003
LAM_C, LAM_R = 0.65, 0.35
THETA0 = 0.5
SPEC_MIN, SPEC_MAX = 0.3, 4.0
B1, B2, EPS = 0.9, 0.999, 1e-8
W_MAX = 2.0 * float(np.sqrt(D))
ALPHA, BETA_H = 0.01, 0.05

Alu = mybir.AluOpType
Act = mybir.ActivationFunctionType
AxX = mybir.AxisListType.X

_BUILT = {}


def _build(general_adam: bool, sim: bool = False):
    key = (general_adam, sim)
    if key in _BUILT:
        return _BUILT[key]
    nc = bacc.Bacc("TRN2", target_bir_lowering=False, debug=False, num_devices=NC)

    def inp(name, shape):
        return nc.declare_dram_parameter(name, list(shape), FP, isOutput=False)

    def outp(name, shape):
        return nc.declare_dram_parameter(name, list(shape), FP, isOutput=True)

    yh_d = inp("yh", (B, D))
    ys_d = inp("ys", (B, D))
    ctr_d = inp("ctr", (NL * F * B, D))        # contribs shard, row (n,f,b)
    vin_d = inp("vin", (NL * B, D))
    vout_d = inp("vout", (NL * B, D))
    vw_d = inp("vw", (NL * B, D))
    a_d = inp("amat", (NL, D, R))
    b_d = inp("bmat", (NL, D, R))
    if general_adam:
        ma_d = inp("ma", (NL, D, R))
        va_d = inp("va", (NL, D, R))
        mb_d = inp("mb", (NL, D, R))
        vb_d = inp("vb", (NL, D, R))
    gd_d = inp("gd", (NL, 1))
    rho_d = inp("rhoi", (1, NL))
    rdot_d = inp("rdoti", (1, NL))
    s_d = inp("smat", (NL, D))
    # host-prepared constants, packed into two blobs (2 DMAs total)
    blobA_d = inp("blobA", (128, 145))
    blobB_d = inp("blobB", (64, 1147))

    anew_d = outp("anew", (NL, D, R))
    bnew_d = outp("bnew", (NL, D, R))
    tens_d = outp("tens", (NL, B, D))
    snew_d = outp("snew", (NL, D))
    rhoo_d = outp("rhoo", (1, NL))
    mse_d = outp("mseo", (1, 1))
    dbgc_d = outp("dbgc", (N, 1))
    dbgu_d = outp("dbgu", (NL, D))

    cc_in = nc.dram_tensor("cc_in", [NL, 5], FP)
    cc_out = nc.dram_tensor("cc_out", [N, 5], FP, addr_space="Shared")

    with tile.TileContext(nc) as tc, ExitStack() as ctx:
        cpool = ctx.enter_context(tc.tile_pool(name="consts", bufs=1))
        main = ctx.enter_context(tc.tile_pool(name="main", bufs=1))
        strm = ctx.enter_context(tc.tile_pool(name="strm", bufs=3))
        scr = ctx.enter_context(tc.tile_pool(name="scr", bufs=3))
        sm = ctx.enter_context(tc.tile_pool(name="small", bufs=2))
        spool = ctx.enter_context(tc.tile_pool(name="spec", bufs=1))
        # PSUM: 8 banks total. pb:2 + pm:2 + pv:3 + acc:1 = 8.
        ppb = ctx.enter_context(tc.tile_pool(name="ppb", bufs=2, space="PSUM"))
        ppm = ctx.enter_context(tc.tile_pool(name="ppm", bufs=3, space="PSUM"))
        ppv = ctx.enter_context(tc.tile_pool(name="ppv", bufs=2, space="PSUM"))
        pacc = ctx.enter_context(tc.tile_pool(name="pacc", bufs=1, space="PSUM"))

        def pb(shape):
            return ppb.tile(list(shape), FP, tag="pb", name="pbt")

        def pm(shape):
            return ppm.tile(list(shape), FP, tag="pm", name="pmt")

        def pv(shape):
            return ppv.tile(list(shape), FP, tag="pv", name="pvt")

        blobA = cpool.tile([128, 145], FP, tag="blobA")
        nc.scalar.dma_start(blobA[:], blobA_d[:])
        blobB = cpool.tile([64, 1147], FP, tag="blobB")
        nc.scalar.dma_start(blobB[:], blobB_d[:])
        ident = blobA[:, 0:128]
        selv8 = [blobA[:, 128:136], blobA[:, 136:144]]
        o128 = blobA[:, 144:145]
        eupT = blobB[:, 0:256]
        elowT = blobB[:, 256:512]
        seln = blobB[:, 512:520]
        e63 = blobB[:, 520:521]
        o64c = blobB[:, 521:522]
        o1_64 = blobB[0:1, 522:586]
        o1_32 = blobB[0:1, 586:618]
        selb = blobB[0:32, 618:626]
        o32a = blobB[0:32, 626:627]
        o32b = blobB[0:32, 627:628]
        mask8 = blobB[0:NL, 628:632]
        lcm = blobB[0:NL, 632:633]
        lrm = blobB[0:NL, 633:634]
        nm8 = blobB[0:NL, 634:635]
        blockmask = blobB[0:NL, 635:1147]

        def tp(out_ps, in_sb):
            k = in_sb.partition_size()
            b = in_sb.base_partition()
            nc.tensor.transpose(out_ps, in_sb, ident[b:b + k, b:b + k])

        # A|B blocks resident in SBUF: abblk[t] is (128, 1024),
        # node n occupies cols [128n,128n+128) as [B_n(64) | A_n(64)].
        abblk = []
        for t in range(8):
            tl = main.tile([128, 1024], FP, tag=f"ab{t}")
            pair = tl[:].rearrange("p (n c) -> p n c", n=NL)
            src_b = b_d[:, 128 * t:128 * (t + 1), :].transpose([1, 0, 2])
            src_a = a_d[:, 128 * t:128 * (t + 1), :].transpose([1, 0, 2])
            nc.sync.dma_start(pair[:, :, 0:64], src_b)
            nc.sync.dma_start(pair[:, :, 64:128], src_a)
            abblk.append(tl)

        def acol(t, n):
            return abblk[t][:, 128 * n + 64:128 * n + 128]

        def bcol(t, n):
            return abblk[t][:, 128 * n:128 * n + 64]

        def abpair(t, n):
            return abblk[t][:, 128 * n:128 * n + 128]

        # ---- phase 1: diff / mse / Tbar / Tl_base -------------------------
        yh = scr.tile([B, D], FP, tag="scratch")
        ys = scr.tile([B, D], FP, tag="scratch")
        nc.scalar.dma_start(yh[:], yh_d[:])
        nc.scalar.dma_start(ys[:], ys_d[:])
        diff = main.tile([B, D], FP, tag="diff")
        nc.vector.tensor_tensor(diff[:], ys[:], yh[:], Alu.subtract)
        dsq = scr.tile([B, D], FP, tag="scratch")
        dacc = sm.tile([B, 1], FP, tag="dacc")
        nc.scalar.activation(dsq[:], diff[:], Act.Square, accum_out=dacc[:])
        mse_ps = pv((1, 1))
        nc.tensor.matmul(mse_ps[:], dacc[:], o32a[:])
        mse_sb = sm.tile([1, 1], FP, tag="mse_sb")
        nc.vector.tensor_copy(mse_sb[:], mse_ps[:])
        nc.sync.dma_start(mse_d[:], mse_sb[:])
        sqc = sm.tile([B, 1], FP, tag="sqc")
        nc.scalar.activation(sqc[:], dacc[:], Act.Sqrt)
        tl_ps = pv((1, 1))
        nc.tensor.matmul(tl_ps[:], sqc[:], o32b[:])
        tl_sb = sm.tile([1, 1], FP, tag="tl_sb")     # Tl_base
        nc.vector.tensor_copy(tl_sb[:], tl_ps[:])
        tnraw = sm.tile([1, 1], FP, tag="tnraw")
        nc.scalar.activation(tnraw[:], mse_sb[:], Act.Sqrt)
        tn = sm.tile([1, 1], FP, tag="tn")           # T_norm
        nc.vector.tensor_scalar_min(tn[:], tnraw[:], 1.0)
        tbar8 = main.tile([NL, D], FP, tag="tbar8")  # Tbar replicated x8
        for ch in range(2):
            cols = slice(512 * ch, 512 * (ch + 1))
            tb_ps = pb((NL, 512))
            nc.tensor.matmul(tb_ps[:], selb[:], diff[:, cols])
            nc.scalar.copy(tbar8[:, cols], tb_ps[:])

        # ---- phase 2: contribution norms ---------------------------------
        sumsqs = main.tile([128, NL], FP, tag="sumsqs")
        for n in range(NL):
            ctile = strm.tile([128, D], FP, tag="ctile", bufs=2)
            nc.scalar.dma_start(ctile[:], ctr_d[128 * n:128 * (n + 1), :])
            csq = scr.tile([128, D], FP, tag="scratch")
            nc.scalar.activation(csq[:], ctile[:], Act.Square,
                                 accum_out=sumsqs[:, n:n + 1])
        nrmc = main.tile([128, NL], FP, tag="nrmc")
        nc.scalar.activation(nrmc[:], sumsqs[:], Act.Sqrt)
        nrmt_ps = pm((NL, 128))
        tp(nrmt_ps[:], nrmc[:])
        nrm8 = main.tile([NL, 128], FP, tag="nrm8")
        nc.vector.tensor_copy(nrm8[:], nrmt_ps[:])
        ccsb = main.tile([NL, 5], FP, tag="ccsb")
        red = sm.tile([NL, F], FP, tag="red")
        nc.vector.tensor_reduce(red[:], nrm8[:].rearrange("p (f b) -> p f b", f=F),
                                axis=AxX, op=Alu.add)
        nc.vector.tensor_tensor(ccsb[:, 0:4], red[:], mask8[:], Alu.mult)

        # ---- phase 3: Grams + spectral norm (node-batched tiles) ---------
        # all per-node (64,64) matrices live side-by-side in (64,512) tiles;
        # each level: 8 matmuls into one PSUM bank + ONE copy out.
        def mcol(tile_, n):
            return tile_[:, 64 * n:64 * (n + 1)]

        # pair-Gram: ONE matmul [B|A]^T [B|A] per (node, t): G_B sits at
        # rows 0:64 cols 0:64, G_A at rows 64:128 cols 64:128 (G_A is
        # extracted to base-0 partitions via PE transpose; exact symmetry).
        ga = spool.tile([N, 512], FP, tag="ga")
        gb = spool.tile([N, 512], FP, tag="gb")
        gatmp = spool.tile([128, 256], FP, tag="gatmp")
        p0 = spool.tile([N, NL], FP, tag="p0")
        p0_ps = pv((N, NL))
        for half in range(2):
            pair_ps = pm((128, 512))
            for k in range(4):
                n = 4 * half + k
                for t in range(8):
                    nc.tensor.matmul(pair_ps[:, 128 * k:128 * (k + 1)],
                                     abpair(t, n), abpair(t, n),
                                     start=(t == 0), stop=(t == 7))
            dstb = gb[:, 256 * half:256 * (half + 1)].rearrange(
                "p (k r) -> p k r", k=4)
            nc.any.tensor_copy(
                dstb, pair_ps[0:64].rearrange("p (k c) -> p k c", k=4)[:, :, 0:64])
            nc.any.tensor_copy(
                gatmp[64:128, :].rearrange("p (k r) -> p k r", k=4),
                pair_ps[64:128].rearrange("p (k c) -> p k c", k=4)[:, :, 64:128])
            gat_ps = pm((N, 256))
            for k in range(4):
                tp(gat_ps[:, 64 * k:64 * (k + 1)],
                   gatmp[64:128, 64 * k:64 * (k + 1)])
            nc.any.tensor_copy(ga[:, 256 * half:256 * (half + 1)], gat_ps[:])
        for n in range(NL):
            for t in range(8):
                nc.tensor.matmul(p0_ps[:, n:n + 1], bcol(t, n), o128,
                                 start=(t == 0), stop=(t == 7))
        nc.vector.tensor_copy(p0[:], p0_ps[:])

        hT_ps = pm((N, 512))
        for n in range(NL):
            nc.tensor.matmul(mcol(hT_ps, n), mcol(ga, n), mcol(gb, n))
        # batched trace normalization: 1/(trace_n/8 + tiny)
        dm = spool.tile([N, 512], FP, tag="dm")
        nc.vector.tensor_tensor(
            dm[:].rearrange("p (n r) -> p n r", n=NL),
            hT_ps[:].rearrange("p (n r) -> p n r", n=NL),
            ident[0:N, 0:N].unsqueeze(1).broadcast_to((N, NL, N)), Alu.mult)
        dred = spool.tile([N, NL], FP, tag="dred")
        nc.vector.tensor_reduce(dred[:], dm[:].rearrange("p (n r) -> p n r", n=NL),
                                axis=AxX, op=Alu.add)
        tr_ps = pv((1, NL))
        nc.tensor.matmul(tr_ps[:], o64c, dred[:])
        tr_row = sm.tile([1, NL], FP, tag="tr_row")
        nc.vector.tensor_scalar_add(tr_row[:], tr_ps[:], 1e-30)
        rec_row = sm.tile([1, NL], FP, tag="rec_row")
        nc.vector.reciprocal(rec_row[:], tr_row[:])
        recb_ps = pv((N, NL))
        nc.tensor.matmul(recb_ps[:], o1_64[:], rec_row[:])
        recB = spool.tile([N, NL], FP, tag="recB")
        nc.vector.tensor_copy(recB[:], recb_ps[:])
        h1T = spool.tile([N, 512], FP, tag="h1T")
        nc.vector.tensor_tensor(
            h1T[:].rearrange("p (n r) -> p n r", n=NL),
            hT_ps[:].rearrange("p (n r) -> p n r", n=NL),
            recB[:].unsqueeze(2).broadcast_to((N, NL, N)), Alu.mult)

        h1P_ps = pm((N, 512))
        for n in range(NL):
            tp(mcol(h1P_ps, n), mcol(h1T, n))
        h1P = spool.tile([N, 512], FP, tag="h1P")
        nc.scalar.copy(h1P[:], h1P_ps[:])

        def sq_level(Ps, Ts, tagT, tagP, need_p=True):
            t_ps = pm((N, 512))
            for n in range(NL):
                nc.tensor.matmul(mcol(t_ps, n), mcol(Ps, n), mcol(Ts, n))
            Tn = spool.tile([N, 512], FP, tag=tagT, name=tagT)
            nc.scalar.copy(Tn[:], t_ps[:])
            Pn = None
            if need_p:
                p_ps = pm((N, 512))
                for n in range(NL):
                    nc.tensor.matmul(mcol(p_ps, n), mcol(Ts, n), mcol(Ps, n))
                Pn = spool.tile([N, 512], FP, tag=tagP, name=tagP)
                nc.scalar.copy(Pn[:], p_ps[:])
            return Tn, Pn

        h2T, h2P = sq_level(h1P, h1T, "h2T", "h2P")
        h4T, h4P = sq_level(h2P, h2T, "h4T", "h4P")
        h8T, _ = sq_level(h4P, h4T, "h8T", "h8P", need_p=False)

        def matvec_level(mats, vecs, tag):
            mv_ps = pv((N, NL))
            for n in range(NL):
                nc.tensor.matmul(mv_ps[:, n:n + 1], mcol(mats, n), vecs[:, n:n + 1])
            out = spool.tile([N, NL], FP, tag=tag, name=tag)
            nc.vector.tensor_copy(out[:], mv_ps[:])
            return out

        v1 = matvec_level(h1T, p0, "v1")
        v2 = matvec_level(h2T, v1, "v2")
        v3 = matvec_level(h8T, v2, "v3")
        qv = matvec_level(ga, v3, "qv")
        rv = matvec_level(gb, qv, "rv")
        s1v = matvec_level(ga, rv, "s1v")
        nd_ps = pv((1, 2 * NL))
        for n in range(NL):
            nc.tensor.matmul(nd_ps[0:1, 2 * n:2 * n + 1], rv[:, n:n + 1], s1v[:, n:n + 1])
            nc.tensor.matmul(nd_ps[0:1, 2 * n + 1:2 * n + 2], qv[:, n:n + 1], rv[:, n:n + 1])
        numden = sm.tile([1, 2 * NL], FP, tag="numden")
        nc.vector.tensor_copy(numden[:], nd_ps[:])

        denr = sm.tile([1, NL], FP, tag="denr")
        nc.vector.tensor_scalar_add(denr[:], numden[:, 1::2], 1e-38)
        denri = sm.tile([1, NL], FP, tag="denri")
        nc.vector.reciprocal(denri[:], denr[:])
        sig2 = sm.tile([1, NL], FP, tag="sig2")
        nc.vector.tensor_tensor(sig2[:], numden[:, 0::2], denri[:], Alu.mult)
        sig = sm.tile([1, NL], FP, tag="sig")
        nc.scalar.activation(sig[:], sig2[:], Act.Sqrt)
        spec_row = sm.tile([1, NL], FP, tag="spec_row")
        nc.vector.tensor_scalar(spec_row[:], sig[:], SPEC_MIN, SPEC_MAX,
                                op0=Alu.max, op1=Alu.min)
        spec_ps = pv((NL, 1))
        tp(spec_ps[:], spec_row[:])
        nc.vector.tensor_copy(ccsb[:, 4:5], spec_ps[:])

        # ---- phase 4: AllGather ------------------------------------------
        nc.gpsimd.dma_start(cc_in[:], ccsb[:])
        if sim:
            # single-core TimelineSim build: stand in for the collective
            nc.gpsimd.dma_start(cc_out[0:NL], cc_in[:])
        else:
            nc.gpsimd.collective_compute(
                "AllGather", Alu.bypass, replica_groups=[list(range(NC))],
                ins=[cc_in[:]], outs=[cc_out[:]])
        gath = main.tile([N, 5], FP, tag="gath")
        nc.gpsimd.dma_start(gath[:], cc_out[:])

        # ---- phase 5: edge weights + coefficient chain -------------------
        den4 = sm.tile([N, 1], FP, tag="den4")
        nc.vector.tensor_reduce(den4[:], gath[:, 0:4], axis=AxX, op=Alu.add)
        den4e = sm.tile([N, 1], FP, tag="den4e")
        nc.vector.tensor_scalar_add(den4e[:], den4[:], 1e-9)
        recw = sm.tile([N, 1], FP, tag="recw")
        nc.vector.reciprocal(recw[:], den4e[:])
        wsp = sm.tile([N, F], FP, tag="wsp")
        nc.vector.tensor_scalar(wsp[:], gath[:, 0:4], recw[:], gath[:, 4:5],
                                op0=Alu.mult, op1=Alu.mult)

        def msum(esrc, tag):
            # w_all[i, f, t] = E[i, f, t] * wsp[i, f]; then sum over f via a
            # strided X-reduce (innermost dim = f, stride N)
            w_all = sm.tile([N, F * N], FP, tag="msw", name="msw", bufs=2)
            nc.vector.tensor_tensor(
                w_all[:].rearrange("p (f t) -> p f t", f=F),
                esrc[:].rearrange("p (f t) -> p f t", f=F),
                wsp[:].unsqueeze(2).broadcast_to((N, F, N)), Alu.mult)
            acc = sm.tile([N, N], FP, tag=f"m_{tag}")
            wv = bass.AP(w_all.tensor, w_all.offset,
                         [list(w_all.ap[0]), [1, N], [N, F]])
            nc.vector.tensor_reduce(acc[:], wv, axis=AxX, op=Alu.add)
            return acc

        mtup = msum(eupT, "up")      # (M_up)^T
        mtlow = msum(elowT, "low")   # (M_low)^T
        mup_ps = pm((N, N))
        tp(mup_ps[:], mtup[:])
        mup = sm.tile([N, N], FP, tag="mup")
        nc.scalar.copy(mup[:], mup_ps[:])

        # power series: v <- v + M^(2^j) v, with (P=M^k, T=(M^k)^T) pairs
        vcur = e63
        Pj, Tj = mup, mtup
        for j in range(6):
            mv_ps = pv((N, 1))
            nc.tensor.matmul(mv_ps[:], Tj[:], vcur[:])
            vnew = sm.tile([N, 1], FP, tag="cv", name="cv", bufs=2)
            nc.vector.tensor_tensor(vnew[:], mv_ps[:], vcur[:], Alu.add)
            vcur = vnew
            if j < 5:
                t_ps = pm((N, N))
                nc.tensor.matmul(t_ps[:], Pj[:], Tj[:])
                Tn = sm.tile([N, N], FP, tag="TpX", name="TpX", bufs=2)
                nc.scalar.copy(Tn[:], t_ps[:])
                if j < 4:
                    p_ps = pm((N, N))
                    nc.tensor.matmul(p_ps[:], Tj[:], Pj[:])
                    Pn = sm.tile([N, N], FP, tag="PpX", name="PpX", bufs=2)
                    nc.scalar.copy(Pn[:], p_ps[:])
                else:
                    Pn = None
                Pj, Tj = Pn, Tn
        low_ps = pv((N, 1))
        nc.tensor.matmul(low_ps[:], mtlow[:], vcur[:])
        c_sb = main.tile([N, 1], FP, tag="c_sb")
        nc.vector.tensor_tensor(c_sb[:], low_ps[:], vcur[:], Alu.add)
        nc.sync.dma_start(dbgc_d[:], c_sb[:])

        crow_ps = pv((1, NL))
        nc.tensor.matmul(crow_ps[:], c_sb[:], seln[:])
        c_row = sm.tile([1, NL], FP, tag="c_row")
        nc.vector.tensor_copy(c_row[:], crow_ps[:])
        cl_ps = pv((NL, 1))
        tp(cl_ps[:], c_row[:])
        c_loc = sm.tile([NL, 1], FP, tag="c_loc")
        nc.vector.tensor_copy(c_loc[:], cl_ps[:])
        cb_ps = pv((B, NL))
        nc.tensor.matmul(cb_ps[:], o1_32[:], c_row[:])
        cb = main.tile([B, NL], FP, tag="cb")
        nc.vector.tensor_copy(cb[:], cb_ps[:])

        # ---- phase 6: tensions out ---------------------------------------
        for n in range(NL):
            tt = strm.tile([B, D], FP, tag="ttile", bufs=2)
            nc.scalar.mul(tt[:], diff[:], cb[:, n:n + 1])
            nc.sync.dma_start(tens_d[n], tt[:])

        # ---- phase 7: health (rho) ---------------------------------------
        tloc = sm.tile([1, NL], FP, tag="tloc")
        nc.vector.tensor_scalar(tloc[:], c_row[:], tl_sb[0:1, 0:1], None,
                                op0=Alu.mult)
        tne = sm.tile([1, 1], FP, tag="tne")
        nc.vector.tensor_scalar_add(tne[:], tn[:], 1e-9)
        tnr = sm.tile([1, 1], FP, tag="tnr")
        nc.vector.reciprocal(tnr[:], tne[:])
        wb_pre = sm.tile([1, NL], FP, tag="wb_pre")
        nc.vector.tensor_scalar(wb_pre[:], tloc[:], tnr[0:1, 0:1], 1.0,
                                op0=Alu.mult, op1=Alu.min)
        fl = sm.tile([1, 1], FP, tag="fl")
        nc.vector.tensor_scalar(fl[:], tn[:], 0.0, None, op0=Alu.is_gt)
        wbm1 = sm.tile([1, NL], FP, tag="wbm1")
        nc.vector.tensor_scalar(wbm1[:], wb_pre[:], 1.0, None, op0=Alu.subtract)
        wbf = sm.tile([1, NL], FP, tag="wbf")
        nc.vector.tensor_scalar(wbf[:], wbm1[:], fl[0:1, 0:1], 1.0,
                                op0=Alu.mult, op1=Alu.add)
        k1 = sm.tile([1, 1], FP, tag="k1")
        nc.vector.tensor_scalar(k1[:], tn[:], -ALPHA, ALPHA, op0=Alu.mult,
                                op1=Alu.add)
        tsq = sm.tile([1, 1], FP, tag="tsq")
        nc.vector.tensor_tensor(tsq[:], tn[:], tn[:], Alu.mult)
        k2a = sm.tile([1, 1], FP, tag="k2a")
        nc.vector.tensor_scalar_add(k2a[:], tsq[:], 1.0)
        k2b = sm.tile([1, 1], FP, tag="k2b")
        nc.vector.tensor_tensor(k2b[:], k2a[:], tn[:], Alu.mult)
        k2 = sm.tile([1, 1], FP, tag="k2")
        nc.vector.tensor_scalar_mul(k2[:], k2b[:], BETA_H)
        negd = sm.tile([1, NL], FP, tag="negd")
        nc.vector.scalar_tensor_tensor(negd[:], wbf[:], k2[0:1, 0:1],
                                       k1[0:1, 0:1].broadcast_to((1, NL)),
                                       op0=Alu.mult, op1=Alu.subtract)
        deltac = sm.tile([1, NL], FP, tag="deltac")
        nc.vector.tensor_scalar(deltac[:], negd[:], -1.0, None, op0=Alu.mult)
        deltac2 = sm.tile([1, NL], FP, tag="deltac2")
        nc.vector.tensor_scalar(deltac2[:], deltac[:], 0.1, -0.1, op0=Alu.min,
                                op1=Alu.max)
        rho_t = sm.tile([1, NL], FP, tag="rho_t")
        nc.sync.dma_start(rho_t[:], rho_d[:])
        rho_n = sm.tile([1, NL], FP, tag="rho_n")
        nc.vector.tensor_tensor(rho_n[:], rho_t[:], deltac2[:], Alu.add)
        rho_c = sm.tile([1, NL], FP, tag="rho_c")
        nc.vector.tensor_scalar(rho_c[:], rho_n[:], 10.0, -5.0, op0=Alu.min,
                                op1=Alu.max)
        nc.sync.dma_start(rhoo_d[:], rho_c[:])

        # ---- phase 8: V statistics ---------------------------------------
        vmr = main.tile([NL, D], FP, tag="vmr")      # mean_b V_in
        vor = main.tile([NL, D], FP, tag="vor")      # mean_b V_out
        vor_q = main.tile([NL, D], FP, tag="vor_q")  # mean_b (V_out t^2)
        t2r = main.tile([NL, D], FP, tag="t2r")      # mean_b t^2
        vth_ps = pacc.tile([NL, 1], FP, tag="pacc")
        for g in range(2):
            vint = main.tile([128, D], FP, tag="vbig", name="vbig", bufs=4)
            voutt = main.tile([128, D], FP, tag="vbig", name="vbig", bufs=4)
            vwt = strm.tile([128, D], FP, tag="ctile", bufs=2)
            rows = slice(128 * g, 128 * (g + 1))
            nc.sync.dma_start(vint[:], vin_d[rows, :])
            nc.sync.dma_start(voutt[:], vout_d[rows, :])
            nc.sync.dma_start(vwt[:], vw_d[rows, :])
            th = scr.tile([128, D], FP, tag="scratch")
            nc.scalar.activation(th[:], vwt[:], Act.Tanh)
            t2t = main.tile([128, D], FP, tag="vbig", name="vbig", bufs=4)
            nc.vector.tensor_tensor(t2t[:], th[:], th[:], Alu.mult)
            qt = main.tile([128, D], FP, tag="vbig", name="vbig", bufs=4)
            nc.vector.tensor_tensor(qt[:], voutt[:], t2t[:], Alu.mult)
            vsq = scr.tile([128, D], FP, tag="scratch")
            vacc = sm.tile([128, 1], FP, tag=f"vacc{g}")
            nc.scalar.activation(vsq[:], vint[:], Act.Square, accum_out=vacc[:])
            nc.tensor.matmul(vth_ps[:], selv8[g][:], vacc[:], start=(g == 0),
                             stop=(g == 1))
            for dst, srct in ((vmr, vint), (vor, voutt), (vor_q, qt), (t2r, t2t)):
                for ch in range(2):
                    cols = slice(512 * ch, 512 * (ch + 1))
                    red_ps = pb((NL, 512))
                    nc.tensor.matmul(red_ps[:], selv8[g][:], srct[:, cols])
                    if g == 0:
                        nc.scalar.copy(dst[:, cols], red_ps[:])
                    else:
                        nc.vector.tensor_tensor(dst[:, cols], dst[:, cols],
                                                red_ps[:], Alu.add)

        theta = sm.tile([NL, 1], FP, tag="theta")
        nc.vector.tensor_scalar(theta[:], vth_ps[:], 1e-9, THETA0, op0=Alu.add,
                                op1=Alu.mult)
        gd_t = sm.tile([NL, 1], FP, tag="gd_t")
        nc.sync.dma_start(gd_t[:], gd_d[:])
        dgc = sm.tile([NL, 1], FP, tag="dgc")
        nc.vector.tensor_tensor(dgc[:], gd_t[:], theta[:], Alu.subtract)
        lamc_dg = sm.tile([NL, 1], FP, tag="lamc_dg")
        nc.vector.tensor_tensor(lamc_dg[:], dgc[:], lcm[:], Alu.mult)

        # ---- phase 9: u assembly ------------------------------------------
        qbar = scr.tile([NL, D], FP, tag="uscr")
        nc.vector.tensor_tensor(qbar[:], vor[:], vor_q[:], Alu.subtract)
        fpb = scr.tile([NL, D], FP, tag="uscr")
        nc.vector.tensor_scalar(fpb[:], t2r[:], -1.0, 1.0, op0=Alu.mult,
                                op1=Alu.add)
        zt = scr.tile([NL, D], FP, tag="uscr")
        nc.vector.tensor_tensor(zt[:], tbar8[:], fpb[:], Alu.mult)
        ucp = main.tile([NL, D], FP, tag="ucp")      # LAM_C*nmask*dg*qbar
        nc.vector.tensor_scalar(ucp[:], qbar[:], lamc_dg[:], None, op0=Alu.mult)
        ur0 = main.tile([NL, D], FP, tag="ur0")      # LAM_R*nmask*(Tbar.fpb)
        nc.vector.tensor_scalar(ur0[:], zt[:], lrm[:], None, op0=Alu.mult)
        u0 = main.tile([NL, D], FP, tag="u0")        # post-c: ucp + c*ur0
        nc.vector.scalar_tensor_tensor(u0[:], ur0[:], c_loc[:], ucp[:],
                                       op0=Alu.mult, op1=Alu.add)
        usq_s = scr.tile([NL, D], FP, tag="uscr")
        uacc = sm.tile([NL, 1], FP, tag="uacc")
        nc.scalar.activation(usq_s[:], u0[:], Act.Square, accum_out=uacc[:])
        vmsq_s = scr.tile([NL, D], FP, tag="uscr")
        vmacc = sm.tile([NL, 1], FP, tag="vmacc")
        nc.scalar.activation(vmsq_s[:], vmr[:], Act.Square, accum_out=vmacc[:])
        un = sm.tile([NL, 1], FP, tag="un")
        nc.scalar.activation(un[:], uacc[:], Act.Sqrt)
        vn = sm.tile([NL, 1], FP, tag="vn")
        nc.scalar.activation(vn[:], vmacc[:], Act.Sqrt)
        gn = sm.tile([NL, 1], FP, tag="gn")
        nc.vector.tensor_tensor(gn[:], un[:], vn[:], Alu.mult)
        gne = sm.tile([NL, 1], FP, tag="gne")
        nc.vector.tensor_scalar_add(gne[:], gn[:], 1e-12)
        rg = sm.tile([NL, 1], FP, tag="rg")
        nc.vector.reciprocal(rg[:], gne[:])
        s_col = sm.tile([NL, 1], FP, tag="s_col")
        nc.vector.tensor_scalar(s_col[:], rg[:], 5.0, 1.0, op0=Alu.mult,
                                op1=Alu.min)
        u_rows = main.tile([NL, D], FP, tag="u_rows")
        nc.vector.tensor_scalar(u_rows[:], u0[:], s_col[:], None, op0=Alu.mult)
        nc.sync.dma_start(dbgu_d[:], u_rows[:])

        # ---- phase 10: signature (S) --------------------------------------
        rd_t = sm.tile([1, NL], FP, tag="rd_t")
        nc.sync.dma_start(rd_t[:], rdot_d[:])
        rde = sm.tile([1, NL], FP, tag="rde")
        nc.vector.tensor_scalar_mul(rde[:], rd_t[:], ETA_S)
        s_tile = main.tile([NL, D], FP, tag="s_tile")
        nc.sync.dma_start(s_tile[:], s_d[:])
        s2 = main.tile([NL, D], FP, tag="s2")
        for ch in range(2):
            cols = slice(512 * ch, 512 * (ch + 1))
            so_ps = pb((NL, 512))
            nc.tensor.matmul(so_ps[:], rde[:], tbar8[0:1, cols])
            nc.vector.tensor_tensor(s2[:, cols], so_ps[:], s_tile[:, cols],
                                    Alu.add)
        s2sq = scr.tile([NL, D], FP, tag="uscr")
        sacc = sm.tile([NL, 1], FP, tag="sacc")
        nc.scalar.activation(s2sq[:], s2[:], Act.Square, accum_out=sacc[:])
        snrm = sm.tile([NL, 1], FP, tag="snrm")
        nc.scalar.activation(snrm[:], sacc[:], Act.Sqrt)
        srec = sm.tile([NL, 1], FP, tag="srec")
        nc.vector.reciprocal(srec[:], snrm[:])
        sfl = sm.tile([NL, 1], FP, tag="sfl")
        nc.vector.tensor_scalar(sfl[:], snrm[:], 1e-9, None, op0=Alu.is_gt)
        srm1 = sm.tile([NL, 1], FP, tag="srm1")
        nc.vector.tensor_scalar(srm1[:], srec[:], 1.0, None, op0=Alu.subtract)
        sf2 = sm.tile([NL, 1], FP, tag="sf2")
        nc.vector.tensor_tensor(sf2[:], sfl[:], srm1[:], Alu.mult)
        sfin = sm.tile([NL, 1], FP, tag="sfin")
        nc.vector.tensor_scalar_add(sfin[:], sf2[:], 1.0)
        s2n = scr.tile([NL, D], FP, tag="uscr")
        nc.vector.tensor_scalar(s2n[:], s2[:], sfin[:], None, op0=Alu.mult)
        sdlt = scr.tile([NL, D], FP, tag="uscr")
        nc.vector.tensor_tensor(sdlt[:], s2n[:], s_tile[:], Alu.subtract)
        sout = scr.tile([NL, D], FP, tag="uscr")
        nc.vector.scalar_tensor_tensor(sout[:], sdlt[:], nm8[:], s_tile[:],
                                       op0=Alu.mult, op1=Alu.add)
        nc.sync.dma_start(snew_d[:], sout[:])

        # ---- phase 11: transposes + matvecs (pre-collective where possible)
        # uv[t] columns: [0:8]=vm, [8:16]=ucp, [16:24]=ur0  (all c-free)
        uv = []
        for t in range(8):
            uvt = main.tile([128, 24], FP, tag=f"uv{t}")
            for j, rows_src in enumerate((vmr, ucp, ur0)):
                c_ps = pm((128, NL))
                tp(c_ps[:], rows_src[:, 128 * t:128 * (t + 1)])
                nc.vector.tensor_copy(uvt[:, 8 * j:8 * (j + 1)], c_ps[:])
            uv.append(uvt)

        # wa = B^T vm;  wb1 = A^T ucp;  wb2 = A^T ur0   (all c-free)
        watile = main.tile([N, NL], FP, tag="watile")
        wb1tile = main.tile([N, NL], FP, tag="wb1tile")
        wb2tile = main.tile([N, NL], FP, tag="wb2tile")
        wa_ps = pv((N, NL))
        wb1_ps = pv((N, NL))
        wb2_ps = pv((N, NL))
        for n in range(NL):
            for t in range(8):
                st = dict(start=(t == 0), stop=(t == 7))
                nc.tensor.matmul(wa_ps[:, n:n + 1], bcol(t, n),
                                 uv[t][:, n:n + 1], **st)
                nc.tensor.matmul(wb1_ps[:, n:n + 1], acol(t, n),
                                 uv[t][:, 8 + n:9 + n], **st)
                nc.tensor.matmul(wb2_ps[:, n:n + 1], acol(t, n),
                                 uv[t][:, 16 + n:17 + n], **st)
        nc.vector.tensor_copy(watile[:], wa_ps[:])
        nc.vector.tensor_copy(wb1tile[:], wb1_ps[:])
        nc.vector.tensor_copy(wb2tile[:], wb2_ps[:])
        war_ps = pv((NL, N))
        tp(war_ps[:], watile[:])
        wa_rows = main.tile([NL, N], FP, tag="wa_rows")
        nc.vector.tensor_copy(wa_rows[:], war_ps[:])
        wb1r_ps = pv((NL, N))
        tp(wb1r_ps[:], wb1tile[:])
        wb1_rows = main.tile([NL, N], FP, tag="wb1_rows")
        nc.vector.tensor_copy(wb1_rows[:], wb1r_ps[:])
        wb2r_ps = pv((NL, N))
        tp(wb2r_ps[:], wb2tile[:])
        wb2_rows = main.tile([NL, N], FP, tag="wb2_rows")
        nc.vector.tensor_copy(wb2_rows[:], wb2r_ps[:])
        # post-c: wb = s * (wb1 + c*wb2);  wa scaled by s is NOT applied to
        # wa (grad_A = u x wA keeps scaling inside u)
        wb_rows = main.tile([NL, N], FP, tag="wb_rows")
        nc.vector.scalar_tensor_tensor(wb_rows[:], wb2_rows[:], c_loc[:],
                                       wb1_rows[:], op0=Alu.mult, op1=Alu.add)
        wbs_rows = main.tile([NL, N], FP, tag="wbs_rows")
        nc.vector.tensor_scalar(wbs_rows[:], wb_rows[:], s_col[:], None,
                                op0=Alu.mult)

        # block-diagonal row tiles for the PE grad build:
        # bd[k, 64n+r] = [k==n] * w_rows[k, r]
        bd_wa = main.tile([NL, 512], FP, tag="bd_wa")
        nc.vector.tensor_tensor(
            bd_wa[:].rearrange("p (n r) -> p n r", n=NL),
            wa_rows[:].rearrange("p (g r) -> p g r", g=1
                                 ).broadcast_to((NL, NL, 64)),
            blockmask[:].rearrange("p (n r) -> p n r", n=NL), Alu.mult)
        bd_wb = main.tile([NL, 512], FP, tag="bd_wb")
        nc.vector.tensor_tensor(
            bd_wb[:].rearrange("p (n r) -> p n r", n=NL),
            wbs_rows[:].rearrange("p (g r) -> p g r", g=1
                                  ).broadcast_to((NL, NL, 64)),
            blockmask[:].rearrange("p (n r) -> p n r", n=NL), Alu.mult)

        # ---- phase 12: Adam updates ---------------------------------------
        abuf = 2 if general_adam else 3
        for t in range(8):
            for side in range(2):  # 0: A-side (u x wA), 1: B-side (vm x wB)
                vec_rows = u_rows if side == 0 else vmr
                bd = bd_wa if side == 0 else bd_wb
                w_cols = (abblk[t][:].rearrange("p (n c) -> p n c", n=NL)
                          [:, :, 64:128] if side == 0 else
                          abblk[t][:].rearrange("p (n c) -> p n c", n=NL)
                          [:, :, 0:64])
                out_d = anew_d if side == 0 else bnew_d
                g_ps = pm((128, 512))
                nc.tensor.matmul(g_ps[:], vec_rows[:, 128 * t:128 * (t + 1)],
                                 bd[:])
                if general_adam:
                    g = strm.tile([128, 512], FP, tag="g", bufs=2)
                    nc.scalar.copy(g[:], g_ps[:])
                    mtile = strm.tile([128, 512], FP, tag="gA", name="gA", bufs=2)
                    vtile = strm.tile([128, 512], FP, tag="gB", name="gB", bufs=2)
                    m_d_, v_d_ = (ma_d, va_d) if side == 0 else (mb_d, vb_d)
                    nc.sync.dma_start(
                        mtile[:].rearrange("p (n r) -> p n r", n=NL),
                        m_d_[:, 128 * t:128 * (t + 1), :].transpose([1, 0, 2]))
                    nc.sync.dma_start(
                        vtile[:].rearrange("p (n r) -> p n r", n=NL),
                        v_d_[:, 128 * t:128 * (t + 1), :].transpose([1, 0, 2]))
                    mh = strm.tile([128, 512], FP, tag="mh", bufs=2)
                    nc.vector.scalar_tensor_tensor(
                        mh[:], mtile[:], B1 / (1.0 - B1), g[:],
                        op0=Alu.mult, op1=Alu.add)
                    g2 = strm.tile([128, 512], FP, tag="gA", name="gA", bufs=2)
                    nc.vector.tensor_tensor(g2[:], g[:], g[:], Alu.mult)
                    vh = strm.tile([128, 512], FP, tag="g", name="g", bufs=2)
                    nc.vector.scalar_tensor_tensor(
                        vh[:], vtile[:], B2 / (1.0 - B2), g2[:],
                        op0=Alu.mult, op1=Alu.add)
                    sq = strm.tile([128, 512], FP, tag="gA", name="gA", bufs=2)
                    nc.scalar.activation(sq[:], vh[:], Act.Sqrt)
                    den = strm.tile([128, 512], FP, tag="gB", name="gB", bufs=2)
                    nc.vector.tensor_scalar_add(den[:], sq[:], EPS)
                    rec = strm.tile([128, 512], FP, tag="rec", bufs=abuf)
                    nc.vector.reciprocal_approx_fast(rec[:], den[:])
                    u1 = strm.tile([128, 512], FP, tag="u1", bufs=abuf)
                    nc.vector.scalar_tensor_tensor(u1[:], mh[:], -ETA_W, rec[:],
                                                   op0=Alu.mult, op1=Alu.mult)
                else:
                    absg = strm.tile([128, 512], FP, tag="absg", bufs=abuf)
                    nc.scalar.activation(absg[:], g_ps[:], Act.Abs)
                    den = strm.tile([128, 512], FP, tag="den", bufs=abuf)
                    nc.vector.tensor_scalar_add(den[:], absg[:], EPS)
                    rec = strm.tile([128, 512], FP, tag="rec", bufs=abuf)
                    nc.vector.reciprocal_approx_fast(rec[:], den[:])
                    u1 = strm.tile([128, 512], FP, tag="u1", bufs=abuf)
                    nc.vector.scalar_tensor_tensor(u1[:], g_ps[:], -ETA_W,
                                                   rec[:], op0=Alu.mult,
                                                   op1=Alu.mult)
                upd = strm.tile([128, 512], FP, tag="upd", bufs=abuf)
                nc.vector.tensor_tensor(
                    upd[:].rearrange("p (n r) -> p n r", n=NL), u1[:].rearrange(
                        "p (n r) -> p n r", n=NL), w_cols, Alu.add)
                eng = nc.sync if side == 0 else nc.scalar
                eng.dma_start(
                    out_d[:, 128 * t:128 * (t + 1), :].transpose([1, 0, 2]),
                    upd[:].rearrange("p (n r) -> p n r", n=NL))

    nc.compile()
    _BUILT[key] = nc
    return nc


def _consts(core, src_ids, src_mask):
    """Host-prepared constant blobs for one core."""
    f32 = np.float32
    lo = core * NL
    A = np.zeros((128, 145), f32)
    A[:, 0:128] = np.eye(128, dtype=f32)
    for g in range(2):
        for p in range(128):
            A[p, 128 + 8 * g + 4 * g + p // 32] = 0  # placeholder (set below)
    # selv8: [p, n] = 1/32 if n == 4*g + p//32
    for g in range(2):
        blk = np.zeros((128, NL), f32)
        for p in range(128):
            blk[p, 4 * g + p // 32] = 1.0 / 32.0
        A[:, 128 + 8 * g:136 + 8 * g] = blk
    A[:, 144] = 1.0 / 32.0                      # o128 (p0 rhs, v0=1/32)

    Bb = np.zeros((64, 1147), f32)
    eup = np.zeros((N, F * N), f32)
    elow = np.zeros((N, F * N), f32)
    for i in range(N):
        for f in range(F):
            if not src_mask[i, f]:
                continue
            t = int(src_ids[i, f])
            if t < i:
                eup[i, N * f + t] += 1.0
            else:
                elow[i, N * f + t] += 1.0
    Bb[:, 0:256] = eup
    Bb[:, 256:512] = elow
    seln = np.zeros((N, NL), f32)
    for n in range(NL):
        seln[lo + n, n] = 1.0
    Bb[:, 512:520] = seln
    Bb[N - 1, 520] = 1.0                        # e63
    Bb[:, 521] = 1.0 / 8.0                      # o64c (trace/8)
    Bb[0, 522:586] = 1.0                        # o1_64
    Bb[0, 586:618] = 1.0 / 32.0                 # o1_32 (tensions bcast, /32)
    Bb[0:32, 618:626] = 1.0 / 1024.0            # selb (Tbar lhsT)
    Bb[0:32, 626] = 1.0 / 32768.0               # o32a (mse)
    Bb[0:32, 627] = 1.0 / 1024.0                # o32b (Tl_base)
    Bb[0:NL, 628:632] = src_mask[lo:lo + NL].astype(f32) / 32.0   # mask8
    nmask = (np.arange(lo, lo + NL) > 0).astype(f32)
    Bb[0:NL, 632] = LAM_C * nmask               # lcm
    Bb[0:NL, 633] = LAM_R * nmask               # lrm
    Bb[0:NL, 634] = nmask                       # nm8
    bm = np.zeros((NL, 512), f32)
    for n in range(NL):
        bm[n, 64 * n:64 * (n + 1)] = 1.0
    Bb[0:NL, 635:1147] = bm
    return {"blobA": A, "blobB": Bb}


def kernel(Y_hat, Y_star, contribs, V_in, V_out, V_weighted, goodness,
           A, B_w, m_A, v_A, m_B, v_B, S, rho, r_dot, src_ids, src_mask):
    f32 = np.float32
    Y_hat = np.ascontiguousarray(Y_hat, f32)
    Y_star = np.ascontiguousarray(Y_star, f32)
    contribs = np.ascontiguousarray(contribs, f32)
    V_in = np.ascontiguousarray(V_in, f32)
    V_out = np.ascontiguousarray(V_out, f32)
    V_weighted = np.ascontiguousarray(V_weighted, f32)
    goodness = np.ascontiguousarray(goodness, f32)
    A = np.ascontiguousarray(A, f32)
    B_w = np.ascontiguousarray(B_w, f32)
    S = np.ascontiguousarray(S, f32)
    rho = np.ascontiguousarray(rho, f32)
    r_dot = np.ascontiguousarray(r_dot, f32)
    src_ids = np.asarray(src_ids)
    src_mask = np.asarray(src_mask)

    general = any(bool(np.any(np.asarray(x))) for x in (m_A, v_A, m_B, v_B))
    nc = _build(general)

    in_maps = []
    for c in range(NC):
        lo = c * NL
        m = {
            "yh": Y_hat, "ys": Y_star,
            "ctr": contribs[lo:lo + NL].reshape(NL * F * B, D),
            "vin": V_in[lo:lo + NL].reshape(NL * B, D),
            "vout": V_out[lo:lo + NL].reshape(NL * B, D),
            "vw": V_weighted[lo:lo + NL].reshape(NL * B, D),
            "amat": A[lo:lo + NL], "bmat": B_w[lo:lo + NL],
            "gd": goodness[lo:lo + NL].reshape(NL, 1),
            "rhoi": rho[lo:lo + NL].reshape(1, NL),
            "rdoti": r_dot[lo:lo + NL].reshape(1, NL),
            "smat": S[lo:lo + NL],
        }
        if general:
            m["ma"] = m_A[lo:lo + NL]
            m["va"] = v_A[lo:lo + NL]
            m["mb"] = m_B[lo:lo + NL]
            m["vb"] = v_B[lo:lo + NL]
        m.update(_consts(c, src_ids, src_mask))
        in_maps.append({k: np.ascontiguousarray(v, f32) for k, v in m.items()})

    res = run_bass_kernel_spmd(nc, in_maps, list(range(NC)))
    outs = res.results

    mse = f32(outs[0]["mseo"][0, 0])
    A_new = np.concatenate([outs[c]["anew"] for c in range(NC)], 0)
    B_new = np.concatenate([outs[c]["bnew"] for c in range(NC)], 0)
    S_new = np.concatenate([outs[c]["snew"] for c in range(NC)], 0)
    rho_new = np.concatenate([outs[c]["rhoo"][0] for c in range(NC)], 0)
    tensions = np.concatenate([outs[c]["tens"] for c in range(NC)], 0)
    # clip is a no-op unless |A - eta*t| exceeds W_MAX; equivalent on host
    if np.abs(A_new).max() > W_MAX or np.abs(B_new).max() > W_MAX:
        np.clip(A_new, -W_MAX, W_MAX, out=A_new)
        np.clip(B_new, -W_MAX, W_MAX, out=B_new)
    return mse, A_new, B_new, S_new, rho_new, tensions
